# revision 41
# baseline (speedup 1.0000x reference)
"""BloomGuidedCrossAttention Trainium2 kernel.

Data-parallel over batch: 8 batch elements -> 8 NeuronCores, weights
replicated.  Per-core Bass/Tile program computes the full cross-attention
block (QKV proj, bloom-bias MLP, biased softmax, context, out-proj,
residual + LayerNorm) for one batch element.

Key design points:
  * All big matmuls run in float32r (TF32 datapath, 1 row/cycle at N>=256
    vs 4 for plain fp32).
  * Q^T/K^T are built head-sliced with an augmented 65th row
    (Q' = [sc*Q, 1], K' = [K, bias_row]) so the PE produces
    sc*scores + bias directly in BOTH orientations - no broadcast adds.
  * exp on ACT; softmax denominators fall out of the q-layout exp via
    accum_out.  Attention probs are written to DRAM from the q-layout;
    context is accumulated from the k-layout P^T (unnormalized), with
    per-head 1/denom folded into the ctx PSUM->SBUF move via a broadcast
    recip row (built with a tiny PE transpose + DRAM-bounce broadcast DMA).
  * out-proj + residual + LayerNorm fused with scalar_tensor_tensor
    (free row-sum accumulators feed the LN statistics).
"""

import sys

sys.path.insert(0, "/opt/trn_rl_repo")

import numpy as np
from contextlib import ExitStack

from concourse import bass, mybir, tile, bass_utils, bacc, masks

F32 = mybir.dt.float32
F32R = mybir.dt.float32r
AF = mybir.ActivationFunctionType
ALU = mybir.AluOpType

S = 1024          # sequence length
D = 768           # d_model
DB = 64           # d_bloom
H = 4             # heads
DH = 192          # head dim
P = 128
NQT = S // P      # 8 query tiles
NKT = S // P      # 8 key tiles
NDC = D // P      # 6 d_model chunks
SC = 1.0 / float(np.sqrt(DH))
LN_EPS = 1e-5

_CACHE = {}


def _emit(nc):
    # ---------------- DRAM I/O ----------------
    x_d = nc.dram_tensor("x", [S, D], F32, kind="ExternalInput").ap()
    bl_d = nc.dram_tensor("bloom", [S, DB], F32, kind="ExternalInput").ap()
    wq_d = nc.dram_tensor("wq", [D, D], F32R, kind="ExternalInput").ap()
    wk_d = nc.dram_tensor("wk", [DB, D], F32R, kind="ExternalInput").ap()
    wv_d = nc.dram_tensor("wv", [D, D], F32R, kind="ExternalInput").ap()
    wo_d = nc.dram_tensor("wo", [D, D], F32R, kind="ExternalInput").ap()
    # bq/bk fed host-side as reshape(4,192).T -> [192, 4] so column h is
    # head h's bias, partition-major.
    bqc_d = nc.dram_tensor("bqc", [DH, H], F32, kind="ExternalInput").ap()
    bkc_d = nc.dram_tensor("bkc", [DH, H], F32, kind="ExternalInput").ap()
    bv_d = nc.dram_tensor("bv", [1, D], F32R, kind="ExternalInput").ap()
    bo_d = nc.dram_tensor("bo", [1, D], F32R, kind="ExternalInput").ap()
    wb1_d = nc.dram_tensor("wb1", [DB, 2 * H], F32R, kind="ExternalInput").ap()
    bb1_d = nc.dram_tensor("bb1", [2 * H, 1], F32, kind="ExternalInput").ap()
    wb2_d = nc.dram_tensor("wb2", [2 * H, H], F32R, kind="ExternalInput").ap()
    bb2_d = nc.dram_tensor("bb2", [H, 1], F32, kind="ExternalInput").ap()
    lng_d = nc.dram_tensor("ln_g", [1, D], F32, kind="ExternalInput").ap()
    lnb_d = nc.dram_tensor("ln_b", [1, D], F32, kind="ExternalInput").ap()

    y_d = nc.dram_tensor("y", [S, D], F32, kind="ExternalOutput").ap()
    at_d = nc.dram_tensor("attn", [H, S, S], F32, kind="ExternalOutput").ap()

    def bcast_dram(dram_ap, parts):
        """DRAM [1, N] -> AP replicating the row across `parts` partitions."""
        return bass.AP(tensor=dram_ap.tensor, offset=dram_ap.offset,
                       ap=[[0, parts]] + dram_ap.ap[1:])

    with tile.TileContext(nc) as tc, ExitStack() as octx:
        glob = octx.enter_context(tc.tile_pool(name="glob", bufs=1))
        dram = octx.enter_context(tc.tile_pool(name="dram", bufs=1,
                                               space="DRAM"))

        ident = glob.tile([P, P], F32, name="ident", tag="ident")
        masks.make_identity(nc, ident[:])

        # per-head ctx^T accumulators (filled in phase B, read in phase C)
        cAs = [glob.tile([P, S], F32R, name=f"cAs{h}", tag=f"cAs{h}")
               for h in range(H)]
        cBs = [glob.tile([DH - P, S], F32R, name=f"cBs{h}", tag=f"cBs{h}")
               for h in range(H)]

        # small constants (A: head dims 0..127, B: head dims 128..191)
        bqcA = glob.tile([P, H], F32, name="bqcA", tag="bqcA")
        bqcB = glob.tile([DH - P, H], F32, name="bqcB", tag="bqcB")
        bkcA = glob.tile([P, H], F32, name="bkcA", tag="bkcA")
        bkcB = glob.tile([DH - P, H], F32, name="bkcB", tag="bkcB")
        nc.sync.dma_start(out=bqcA, in_=bqc_d[0:P, :])
        nc.sync.dma_start(out=bqcB, in_=bqc_d[P:DH, :])
        nc.sync.dma_start(out=bkcA, in_=bkc_d[0:P, :])
        nc.sync.dma_start(out=bkcB, in_=bkc_d[P:DH, :])

        bvr = glob.tile([1, D], F32R, name="bvr", tag="bvr")
        bor = glob.tile([1, D], F32R, name="bor", tag="bor")
        nc.sync.dma_start(out=bvr, in_=bv_d)
        nc.sync.dma_start(out=bor, in_=bo_d)
        bb1c = glob.tile([2 * H, 1], F32, name="bb1c", tag="bb1c")
        bb2c = glob.tile([H, 1], F32, name="bb2c", tag="bb2c")
        nc.sync.dma_start(out=bb1c, in_=bb1_d)
        nc.sync.dma_start(out=bb2c, in_=bb2_d)
        onesf = glob.tile([1, P], F32, name="onesf", tag="onesf")
        nc.vector.memset(onesf, 1.0)
        ones1 = glob.tile([1, P], F32R, name="ones1", tag="ones1")
        nc.scalar.activation(out=ones1[:], in_=onesf[:], func=AF.Copy)



        with ExitStack() as qctx:
            qk = qctx.enter_context(tc.tile_pool(name="qk", bufs=1))
            qtA = [qk.tile([P, S], F32R, name=f"qtA{h}", tag=f"qtA{h}")
                   for h in range(H)]
            qtB = [qk.tile([DH - P + 1, S], F32R, name=f"qtB{h}",
                           tag=f"qtB{h}") for h in range(H)]
            ktA = [qk.tile([P, S], F32R, name=f"ktA{h}", tag=f"ktA{h}")
                   for h in range(H)]
            ktB = [qk.tile([DH - P + 1, S], F32R, name=f"ktB{h}",
                           tag=f"ktB{h}") for h in range(H)]
            vt = [qk.tile([P, D], F32R, name=f"vt{i}", tag=f"vt{i}")
                  for i in range(NQT)]

            # ---------------- Phase A: transposes + projections ----------
            with ExitStack() as actx:
                ap_ = actx.enter_context(tc.tile_pool(name="apool", bufs=1))
                pT = actx.enter_context(
                    tc.tile_pool(name="pT", bufs=2, space="PSUM"))
                pA = actx.enter_context(
                    tc.tile_pool(name="pA", bufs=3, space="PSUM"))

                # bloom^T  [64, 1024]
                blT = ap_.tile([DB, S], F32R, name="blT", tag="blT")
                for g in range(2):          # groups of 4 seq-chunks
                    pt = pT.tile([P, 4 * P], F32, name="pt", tag="pt")
                    for j in range(4):
                        i = g * 4 + j
                        bln = ap_.tile([P, DB], F32, name="bln", tag="bln",
                                       bufs=5)
                        nc.sync.dma_start(out=bln,
                                          in_=bl_d[i * P:(i + 1) * P, :])
                        nc.tensor.transpose(pt[0:DB, j * P:(j + 1) * P],
                                            bln[:], ident[:])
                    nc.scalar.activation(out=blT[:, g * 4 * P:(g + 1) * 4 * P],
                                         in_=pt[0:DB, :], func=AF.Copy)

                # bias MLP: h1^T = gelu(Wb1^T bloom^T + bb1) [8, 1024]
                wb1t = ap_.tile([DB, 2 * H], F32R, name="wb1t", tag="wb1t")
                wb2t = ap_.tile([2 * H, H], F32R, name="wb2t", tag="wb2t")
                nc.sync.dma_start(out=wb1t, in_=wb1_d)
                nc.sync.dma_start(out=wb2t, in_=wb2_d)
                h1t = ap_.tile([2 * H, S], F32R, name="h1t", tag="h1t")
                biasT = ap_.tile([H, S], F32R, name="biasT", tag="biasT")
                for g in range(2):
                    ph = pA.tile([P, D], F32, name="ph", tag="pa")
                    nc.tensor.matmul(ph[0:2 * H, 0:512], wb1t[:],
                                     blT[:, g * 512:(g + 1) * 512],
                                     start=True, stop=True)
                    nc.scalar.activation(out=h1t[:, g * 512:(g + 1) * 512],
                                         in_=ph[0:2 * H, 0:512], func=AF.Gelu,
                                         bias=bb1c[:])
                for g in range(2):
                    ph = pA.tile([P, D], F32, name="ph", tag="pa")
                    nc.tensor.matmul(ph[0:H, 0:512], wb2t[:],
                                     h1t[:, g * 512:(g + 1) * 512],
                                     start=True, stop=True)
                    nc.scalar.activation(out=biasT[:, g * 512:(g + 1) * 512],
                                         in_=ph[0:H, 0:512], func=AF.Identity,
                                         bias=bb2c[:])

                # X^T [768, 1024] as 6 chunk tiles  (X loaded transiently)
                xt = [ap_.tile([P, S], F32R, name=f"xt{c}", tag=f"xt{c}")
                      for c in range(NDC)]
                for g in range(2):
                    xns = []
                    for j in range(4):
                        i = g * 4 + j
                        xn = ap_.tile([P, D], F32, name="xn", tag="xn", bufs=5)
                        nc.sync.dma_start(out=xn,
                                          in_=x_d[i * P:(i + 1) * P, :])
                        xns.append(xn)
                    for c in range(NDC):
                        pt = pT.tile([P, 4 * P], F32, name="pt", tag="pt")
                        for j in range(4):
                            nc.tensor.transpose(
                                pt[:, j * P:(j + 1) * P],
                                xns[j][:, c * P:(c + 1) * P], ident[:])
                        nc.scalar.activation(
                            out=xt[c][:, g * 4 * P:(g + 1) * 4 * P],
                            in_=pt[:], func=AF.Copy)

                # ones row of Q'^T tiles (partition 64), via f32 staging
                tmpB = ap_.tile([DH - P + 1, S], F32, name="tmpB", tag="tmpB")
                nc.vector.memset(tmpB[DH - P:DH - P + 1, :], 1.0)
                for h in range(H):
                    nc.scalar.activation(out=qtB[h][DH - P:DH - P + 1, :],
                                         in_=tmpB[DH - P:DH - P + 1, :],
                                         func=AF.Copy)
                # bias row of K'^T tiles: biasT row h -> ktB[h] partition 64
                for h in range(H):
                    nc.sync.dma_start(out=ktB[h][DH - P:DH - P + 1, :],
                                      in_=biasT[h:h + 1, :])

                # Q^T = sc*(Wq^T X^T + bq) head-sliced; K^T = Wk^T bloom^T+bk
                wbig = [ap_.tile([P, D], F32R, name=f"wbig{c}", tag=f"wbig{c}")
                        for c in range(NDC)]
                for c in range(NDC):
                    nc.sync.dma_start(out=wbig[c],
                                      in_=wq_d[c * P:(c + 1) * P, :])
                for h in range(H):
                    pa = pA.tile([P, S], F32, name="pa", tag="pa")
                    pb = pA.tile([P, S], F32, name="pb", tag="pa")
                    for g in range(2):
                        ql = slice(g * 512, (g + 1) * 512)
                        for c in range(NDC):
                            nc.tensor.matmul(
                                pa[:, ql],
                                wbig[c][:, h * DH:h * DH + P],
                                xt[c][:, ql],
                                start=(c == 0), stop=(c == NDC - 1))
                        for c in range(NDC):
                            nc.tensor.matmul(
                                pb[0:DH - P, ql],
                                wbig[c][:, h * DH + P:(h + 1) * DH],
                                xt[c][:, ql],
                                start=(c == 0), stop=(c == NDC - 1))
                    nc.vector.tensor_scalar(
                        out=qtA[h][:, :], in0=pa[:, :],
                        scalar1=bqcA[:, h:h + 1], scalar2=SC,
                        op0=ALU.add, op1=ALU.mult)
                    nc.vector.tensor_scalar(
                        out=qtB[h][0:DH - P, :], in0=pb[0:DH - P, :],
                        scalar1=bqcB[:, h:h + 1], scalar2=SC,
                        op0=ALU.add, op1=ALU.mult)

                wkt = ap_.tile([DB, D], F32R, name="wkt", tag="wkt")
                nc.sync.dma_start(out=wkt, in_=wk_d)
                for h in range(H):
                    pa = pA.tile([P, S], F32, name="pa2", tag="pa")
                    pb = pA.tile([P, S], F32, name="pb2", tag="pa")
                    for g in range(2):
                        ql = slice(g * 512, (g + 1) * 512)
                        nc.tensor.matmul(pa[:, ql],
                                         wkt[:, h * DH:h * DH + P],
                                         blT[:, ql], start=True, stop=True)
                        nc.tensor.matmul(pb[0:DH - P, ql],
                                         wkt[:, h * DH + P:(h + 1) * DH],
                                         blT[:, ql], start=True, stop=True)
                    nc.vector.tensor_scalar_add(
                        out=ktA[h][:, :], in0=pa[:, :],
                        scalar1=bkcA[:, h:h + 1])
                    nc.vector.tensor_scalar_add(
                        out=ktB[h][0:DH - P, :], in0=pb[0:DH - P, :],
                        scalar1=bkcB[:, h:h + 1])

                # V = X Wv + bv (natural layout), reusing wbig slots
                wv = [ap_.tile([P, D], F32R, name=f"wv{c}", tag=f"wbig{c}")
                      for c in range(NDC)]
                for c in range(NDC):
                    nc.sync.dma_start(out=wv[c],
                                      in_=wv_d[c * P:(c + 1) * P, :])
                for i in range(NQT):
                    pv = pA.tile([P, D], F32, name="pv", tag="pa")
                    for n0, n1 in ((0, 512), (512, D)):
                        for c in range(NDC):
                            nc.tensor.matmul(
                                pv[:, n0:n1],
                                xt[c][:, i * P:(i + 1) * P],
                                wv[c][:, n0:n1],
                                start=(c == 0), stop=False)
                        nc.tensor.matmul(pv[:, n0:n1], ones1[:],
                                         bvr[:, n0:n1],
                                         start=False, stop=True)
                    nc.vector.tensor_copy(out=vt[i][:], in_=pv[:])

            # ---------------- Phase B: attention ----------------
            with ExitStack() as bctx:
                bp = bctx.enter_context(tc.tile_pool(name="bpool", bufs=1))
                pS = bctx.enter_context(
                    tc.tile_pool(name="pS", bufs=2, space="PSUM"))
                pC = bctx.enter_context(
                    tc.tile_pool(name="pC", bufs=1, space="PSUM"))

                for h in range(H):
                    qA, qB, kA, kB = qtA[h], qtB[h], ktA[h], ktB[h]

                    # B.1: q-layout scores, exp + denom (accum), attn out.
                    # Done first so the recip row is ready before ctx moves.
                    rc = bp.tile([P, NQT], F32, name="rc", tag="rc", bufs=2)
                    for qt in range(NQT):
                        qsl = slice(qt * P, (qt + 1) * P)
                        den = bp.tile([P, 1], F32, name="den", tag="den",
                                      bufs=4)
                        ps_ = pS.tile([P, S], F32, name="ps2", tag="ps")
                        for g in range(2):
                            ksl = slice(g * 512, (g + 1) * 512)
                            nc.tensor.matmul(ps_[:, ksl], qA[:, qsl],
                                             kA[:, ksl],
                                             start=True, stop=False)
                            nc.tensor.matmul(ps_[:, ksl], qB[:, qsl],
                                             kB[:, ksl],
                                             start=False, stop=True)
                        st = bp.tile([P, S], F32, name="stg", tag="stg",
                                     bufs=4)
                        nc.scalar.activation(out=st[:], in_=ps_[:],
                                             func=AF.Exp, accum_out=den[:])
                        nc.vector.reciprocal(out=rc[:, qt:qt + 1],
                                             in_=den[:])
                        at_st = bp.tile([P, S], F32, name="at_st",
                                        tag="at_st", bufs=4)
                        nc.gpsimd.tensor_scalar_mul(
                            out=at_st[:], in0=st[:],
                            scalar1=rc[:, qt:qt + 1])
                        nc.sync.dma_start(
                            out=at_d[h, qt * P:(qt + 1) * P, :],
                            in_=at_st[:])

                    # B.2: recip cols -> DRAM (transposing DMA, flat recip[q])
                    # -> broadcast tile.  Overlaps with B.3 below.
                    rscr = dram.tile([NQT, P], F32, name="rscr", tag="rscr",
                                     bufs=2)
                    # element (p, qt) of rc lands at flat[qt*128 + p]
                    rdst = bass.AP(tensor=rscr.tensor, offset=rscr.offset,
                                   ap=[[1, P], [P, NQT]])
                    nc.sync.dma_start(out=rdst, in_=rc[:])
                    rbc = bp.tile([P, S], F32, name="rbc", tag="rbc", bufs=2)
                    rsrc = bass.AP(tensor=rscr.tensor, offset=rscr.offset,
                                   ap=[[0, P], [1, S]])
                    nc.sync.dma_start(out=rbc[:], in_=rsrc)

                    # B.3: S^T -> exp -> P^T chunk tiles [128 k, 1024 q]
                    pts = []
                    for kt in range(NKT):
                        ptile = bp.tile([P, S], F32R, name="ptile",
                                        tag="ptile", bufs=9)
                        ksl = slice(kt * P, (kt + 1) * P)
                        ps_ = pS.tile([P, S], F32, name="ps", tag="ps")
                        for g in range(2):
                            qsl = slice(g * 512, (g + 1) * 512)
                            nc.tensor.matmul(ps_[:, qsl], kA[:, ksl],
                                             qA[:, qsl],
                                             start=True, stop=False)
                            nc.tensor.matmul(ps_[:, qsl], kB[:, ksl],
                                             qB[:, qsl],
                                             start=False, stop=True)
                        nc.scalar.activation(out=ptile[:], in_=ps_[:],
                                             func=AF.Exp)
                        pts.append(ptile)

                    # B.4: ctx^T accumulation (unnormalized)
                    cpa = pC.tile([P, S], F32, name="cpa", tag="cpa")
                    cpb = pC.tile([DH - P, S], F32, name="cpb", tag="cpb")
                    for g in range(2):
                        qsl = slice(g * 512, (g + 1) * 512)
                        for kt in range(NKT):
                            nc.tensor.matmul(
                                cpa[:, qsl],
                                vt[kt][:, h * DH:h * DH + P],
                                pts[kt][:, qsl],
                                start=(kt == 0), stop=(kt == NKT - 1))
                        for kt in range(NKT):
                            nc.tensor.matmul(
                                cpb[:, qsl],
                                vt[kt][:, h * DH + P:(h + 1) * DH],
                                pts[kt][:, qsl],
                                start=(kt == 0), stop=(kt == NKT - 1))

                    # B.5: normalized moves (recip row arrived during B.3)
                    nc.vector.scalar_tensor_tensor(
                        out=cAs[h][:], in0=cpa[:], scalar=0.0, in1=rbc[:],
                        op0=ALU.bypass, op1=ALU.mult)
                    nc.vector.scalar_tensor_tensor(
                        out=cBs[h][:], in0=cpb[:], scalar=0.0,
                        in1=rbc[0:DH - P, :], op0=ALU.bypass, op1=ALU.mult)

        # ---------------- Phase C: out-proj + residual + LN ----------
        with ExitStack() as cctx:
            cp = cctx.enter_context(tc.tile_pool(name="cpool", bufs=1))
            pO = cctx.enter_context(
                tc.tile_pool(name="pO", bufs=2, space="PSUM"))

            woA = [cp.tile([P, D], F32R, name=f"woA{h}", tag=f"woA{h}")
                   for h in range(H)]
            woB = [cp.tile([DH - P, D], F32R, name=f"woB{h}", tag=f"woB{h}")
                   for h in range(H)]
            for h in range(H):
                nc.sync.dma_start(out=woA[h],
                                  in_=wo_d[h * DH:h * DH + P, :])
                nc.sync.dma_start(out=woB[h],
                                  in_=wo_d[h * DH + P:(h + 1) * DH, :])
            gbc = cp.tile([P, D], F32, name="gbc", tag="gbc")
            lbc = cp.tile([P, D], F32, name="lbc", tag="lbc")
            nc.sync.dma_start(out=gbc[:], in_=bcast_dram(lng_d, P))
            nc.sync.dma_start(out=lbc[:], in_=bcast_dram(lnb_d, P))
            epst = cp.tile([P, 1], F32, name="epst", tag="epst")
            nc.vector.memset(epst, LN_EPS)

            # prefetch residual X tiles
            xrs = []
            for qt in range(NQT):
                xr = cp.tile([P, D], F32, name=f"xr{qt}", tag=f"xr{qt}")
                nc.sync.dma_start(out=xr, in_=x_d[qt * P:(qt + 1) * P, :])
                xrs.append(xr)

            for qt in range(NQT):
                qsl = slice(qt * P, (qt + 1) * P)
                po = pO.tile([P, D], F32, name="po", tag="po")
                for n0, n1 in ((0, 512), (512, D)):
                    first = True
                    for h in range(H):
                        nc.tensor.matmul(po[:, n0:n1], cAs[h][:, qsl],
                                         woA[h][:, n0:n1],
                                         start=first, stop=False)
                        first = False
                        nc.tensor.matmul(po[:, n0:n1], cBs[h][:, qsl],
                                         woB[h][:, n0:n1],
                                         start=False, stop=False)
                    nc.tensor.matmul(po[:, n0:n1], ones1[:], bor[:, n0:n1],
                                     start=False, stop=True)
                xr = xrs[qt]
                xres = cp.tile([P, D], F32, name="xres", tag="xres", bufs=3)
                xsum = cp.tile([P, 4], F32, name="xsum", tag="xsum", bufs=3)
                nc.vector.scalar_tensor_tensor(
                    out=xres[:], in0=po[:], scalar=0.0, in1=xr[:],
                    op0=ALU.bypass, op1=ALU.add, accum_out=xsum[:, 0:1])
                # LN stats
                xsq = cp.tile([P, D], F32, name="xsq", tag="xsq", bufs=2)
                nc.scalar.activation(out=xsq[:], in_=xres[:], func=AF.Square,
                                     accum_out=xsum[:, 1:2])
                mu = cp.tile([P, 4], F32, name="mu", tag="mu", bufs=3)
                nc.scalar.mul(out=mu[:, 0:1], in_=xsum[:, 0:1], mul=1.0 / D)
                nc.scalar.mul(out=mu[:, 1:2], in_=xsum[:, 1:2], mul=1.0 / D)
                nc.vector.tensor_mul(out=mu[:, 2:3], in0=mu[:, 0:1],
                                     in1=mu[:, 0:1])
                nc.vector.tensor_sub(out=mu[:, 3:4], in0=mu[:, 1:2],
                                     in1=mu[:, 2:3])   # var
                rstd = cp.tile([P, 2], F32, name="rstd", tag="rstd", bufs=3)
                nc.scalar.activation(out=rstd[:, 0:1], in_=mu[:, 3:4],
                                     func=AF.Sqrt, bias=epst[:])
                nc.vector.reciprocal(out=rstd[:, 1:2], in_=rstd[:, 0:1])
                # y = (x - mu) * rstd * g + b
                t1 = cp.tile([P, D], F32, name="t1", tag="xsq", bufs=2)
                nc.vector.scalar_tensor_tensor(
                    out=t1[:], in0=xres[:], scalar=mu[:, 0:1], in1=gbc[:],
                    op0=ALU.subtract, op1=ALU.mult)
                yt = cp.tile([P, D], F32, name="yt", tag="yt", bufs=3)
                nc.vector.scalar_tensor_tensor(
                    out=yt[:], in0=t1[:], scalar=rstd[:, 1:2], in1=lbc[:],
                    op0=ALU.mult, op1=ALU.add)
                nc.sync.dma_start(out=y_d[qsl, :], in_=yt[:])


def _build():
    if "nc" in _CACHE:
        return _CACHE["nc"]
    nc = bacc.Bacc("TRN2", target_bir_lowering=False, debug=False)
    _emit(nc)
    nc.compile()
    _CACHE["nc"] = nc
    return nc


def kernel(dnabert_hidden, bloom_encoding, Wq, bq, Wk, bk, Wv, bv,
           Wb1, bb1, Wb2, bb2, Wo, bo, ln_g, ln_b):
    nc = _build()
    B = dnabert_hidden.shape[0]
    f = np.ascontiguousarray
    shared = {
        "wq": f(np.asarray(Wq, np.float32)),
        "wk": f(np.asarray(Wk, np.float32)),
        "wv": f(np.asarray(Wv, np.float32)),
        "wo": f(np.asarray(Wo, np.float32)),
        "bqc": f(np.asarray(bq, np.float32).reshape(H, DH).T),
        "bkc": f(np.asarray(bk, np.float32).reshape(H, DH).T),
        "bv": f(np.asarray(bv, np.float32).reshape(1, D)),
        "bo": f(np.asarray(bo, np.float32).reshape(1, D)),
        "wb1": f(np.asarray(Wb1, np.float32)),
        "bb1": f(np.asarray(bb1, np.float32).reshape(2 * H, 1)),
        "wb2": f(np.asarray(Wb2, np.float32)),
        "bb2": f(np.asarray(bb2, np.float32).reshape(H, 1)),
        "ln_g": f(np.asarray(ln_g, np.float32).reshape(1, D)),
        "ln_b": f(np.asarray(ln_b, np.float32).reshape(1, D)),
    }
    in_maps = []
    for b in range(B):
        m = dict(shared)
        m["x"] = f(np.asarray(dnabert_hidden[b], np.float32))
        m["bloom"] = f(np.asarray(bloom_encoding[b], np.float32))
        in_maps.append(m)
    res = bass_utils.run_bass_kernel_spmd(nc, in_maps,
                                          core_ids=list(range(B)))
    y = np.stack([res.results[b]["y"] for b in range(B)])
    attn = np.stack([res.results[b]["attn"] for b in range(B)])
    return y, attn
